# revision 3
# baseline (speedup 1.0000x reference)
"""Bass/Trainium2 kernel for nn_BioConvolution (per-patch unshared-filter conv).

Math: for each of P=256 non-overlapping 8x8 patches p,
    Z[n, p, :] = relu(A_p @ W_p + bias),  A_p = patches[n, p] in R^{64 x 2048},
    W_p in R^{2048 x 128}.

Sharding: patch dimension P across 8 cores (32 patches/core) -- each filter
and each input pixel is read exactly once chip-wide (batch-parallel would
replicate the 256MB filter tensor 8x).

Device layout (host pre-transposes, all contiguous large DMAs):
  A  [128k, 32p, 16kc, 64n]  -- patch pixels, contraction-major
  Wt [128k, 32p, 16kc, 128o] -- filters, contraction-major
  Z  [128o, 32p, 64n]        -- output (host transposes back)
Per patch: 16 accumulating fp32 matmuls (lhsT = W chunk [128k,128o] stationary,
rhs = A chunk [128k,64n] moving) into one PSUM bank, then ScalarE Relu+bias
into SBUF, group-DMA out.
"""

from contextlib import ExitStack

import numpy as np

N_CORES = 8
N, H, W_IMG, FIN = 64, 128, 128, 32
FH = FW = 8
FOUT = 128
NR, NCOL = H // FH, W_IMG // FW  # 16, 16
P = NR * NCOL  # 256
PPC = P // N_CORES  # 32 patches per core
K = FH * FW * FIN  # 2048
KP = 128  # contraction tile (partition dim)
KC = K // KP  # 16 chunks
GP = 4  # patches per DMA group
NG = PPC // GP  # 8 groups

_PROGRAM_CACHE = {}


def build_program(bufs=3, psum_bufs=4):
    import concourse.mybir as mybir
    import concourse.tile as tile
    from concourse import bacc

    nc = bacc.Bacc()
    a_d = nc.dram_tensor("A", [KP, PPC, KC, N], mybir.dt.float32, kind="ExternalInput")
    w_d = nc.dram_tensor(
        "Wt", [KP, PPC, KC, FOUT], mybir.dt.float32, kind="ExternalInput"
    )
    b_d = nc.dram_tensor("bias", [FOUT], mybir.dt.float32, kind="ExternalInput")
    z_d = nc.dram_tensor("Z", [FOUT, PPC, N], mybir.dt.float32, kind="ExternalOutput")

    with tile.TileContext(nc) as tc, ExitStack() as ctx:
        wpool = ctx.enter_context(tc.tile_pool(name="w", bufs=bufs))
        apool = ctx.enter_context(tc.tile_pool(name="a", bufs=bufs))
        opool = ctx.enter_context(tc.tile_pool(name="o", bufs=bufs))
        pspool = ctx.enter_context(tc.tile_pool(name="ps", bufs=psum_bufs, space="PSUM"))
        singles = ctx.enter_context(tc.tile_pool(name="singles", bufs=1))

        bias_sb = singles.tile([FOUT, 1], mybir.dt.float32)
        nc.sync.dma_start(out=bias_sb, in_=b_d[:, None])

        for g in range(NG):
            p0 = g * GP
            wt = wpool.tile([KP, GP, KC, FOUT], mybir.dt.float32)
            nc.sync.dma_start(out=wt, in_=w_d[:, p0 : p0 + GP, :, :])
            at = apool.tile([KP, GP, KC, N], mybir.dt.float32)
            nc.sync.dma_start(out=at, in_=a_d[:, p0 : p0 + GP, :, :])
            ot = opool.tile([FOUT, GP, N], mybir.dt.float32)
            for j in range(GP):
                ps = pspool.tile([FOUT, N], mybir.dt.float32)
                for kc in range(KC):
                    nc.tensor.matmul(
                        ps,
                        wt[:, j, kc, :],
                        at[:, j, kc, :],
                        start=(kc == 0),
                        stop=(kc == KC - 1),
                    )
                nc.scalar.activation(
                    ot[:, j, :],
                    ps,
                    mybir.ActivationFunctionType.Relu,
                    bias=bias_sb,
                )
            nc.sync.dma_start(out=z_d[:, p0 : p0 + GP, :], in_=ot)
    nc.finalize()
    return nc


def shard_inputs(X, filters, bias):
    """Host-side reshape into per-core device layouts (all float32)."""
    X = np.asarray(X, dtype=np.float32)
    filters = np.asarray(filters, dtype=np.float32)
    bias = np.ascontiguousarray(np.asarray(bias, dtype=np.float32))

    # X [n, h, w, c] -> patches [p, g, n] with g = (fh, fw, c) row-major
    xr = X.reshape(N, NR, FH, NCOL, FW, FIN)
    xp = xr.transpose(1, 3, 2, 4, 5, 0).reshape(P, K, N)
    # -> [core, k(128), p_local(32), kc(16), n]
    a_all = np.ascontiguousarray(
        xp.reshape(N_CORES, PPC, KC, KP, N).transpose(0, 3, 1, 2, 4)
    )

    # filters [p, fh, fw, c, o] -> [p, g, o] -> [core, k, p_local, kc, o]
    wp = filters.reshape(P, K, FOUT)
    w_all = np.ascontiguousarray(
        wp.reshape(N_CORES, PPC, KC, KP, FOUT).transpose(0, 3, 1, 2, 4)
    )

    return [
        {"A": a_all[c], "Wt": w_all[c], "bias": bias} for c in range(N_CORES)
    ]


def gather_output(per_core_z):
    """per_core_z: list of 8 arrays [FOUT, PPC, N] -> [N, NR, NCOL, FOUT]."""
    z = np.stack(per_core_z, axis=0)  # [core, o, p, n]
    z = z.transpose(3, 0, 2, 1).reshape(N, P, FOUT)  # [n, p_global, o]
    return np.ascontiguousarray(z.reshape(N, NR, NCOL, FOUT))


def kernel(X, filters, bias):
    from concourse.bass_utils import run_bass_kernel_spmd

    if "nc" not in _PROGRAM_CACHE:
        _PROGRAM_CACHE["nc"] = build_program()
    nc = _PROGRAM_CACHE["nc"]

    in_maps = shard_inputs(X, filters, bias)
    res = run_bass_kernel_spmd(nc, in_maps, core_ids=list(range(N_CORES)))
    return gather_output([res.results[c]["Z"] for c in range(N_CORES)])


# revision 6
# speedup vs baseline: 1.8980x; 1.8980x over previous
"""Bass/Trainium2 kernel for nn_BioConvolution (per-patch unshared-filter conv).

Math: for each of P=256 non-overlapping 8x8 patches p,
    Z[n, p, :] = relu(A_p @ W_p + bias),  A_p = patches[n, p] in R^{64 x 2048},
    W_p in R^{2048 x 128}.

Sharding: patch dimension P across 8 cores (32 patches/core) -- every filter
and input pixel is read exactly once chip-wide (batch-parallel would
replicate the 256MB filter tensor 8x). The kernel is HBM-bound: ~50MB/core.

Precision: fp32 matmul on TRN2 lowers to 2 half-rate LOW_HIGH passes
(~434ns/chunk measured) which makes PE the bottleneck. Instead we use a
split-bf16 scheme: A = Ah + Al, W = Wh + Wl (bf16 each, exact fp32 sum of
products in PSUM): A*W ~= Ah*Wh + Al*Wh + Ah*Wl, dropping only Al*Wl
(~2^-18 relative). Measured rel err ~1e-5 vs fp32 reference. Same DMA bytes
as fp32, but full-rate bf16 matmuls.

Device layout (host pre-transposes; all DMAs are large and contiguous):
  Ax [128k, 32p, 16kc, 2, 64n]  bf16 -- patch pixels (hi/lo), k-major
  Wx [128k, 32p, 16kc, 2, 128o] bf16 -- filters (hi/lo), k-major
  Z  [128o, 32p, 64n] f32 -- output (host transposes back)
Per patch, one PSUM tile [128o, 2, 64n] accumulates 16 chunks x 2 matmuls:
  mmA: lhsT=Wh [128k,128o], rhs=[Ah|Al] [128k,128]  -> psum[:, :, :]
  mmB: lhsT=Wl [128k,128o], rhs=Ah [128k,64]        -> psum[:, 0, :]
Epilogue per patch: DVE add (hi+lo halves) then ScalarE Relu+bias.
"""

from contextlib import ExitStack

import numpy as np

N_CORES = 8
N, H, W_IMG, FIN = 64, 128, 128, 32
FH = FW = 8
FOUT = 128
NR, NCOL = H // FH, W_IMG // FW  # 16, 16
P = NR * NCOL  # 256
PPC = P // N_CORES  # 32 patches per core
K = FH * FW * FIN  # 2048
KP = 128  # contraction tile (partition dim)
KC = K // KP  # 16 chunks
GP = 4  # patches per DMA group
NG = PPC // GP  # 8 groups

_PROGRAM_CACHE = {}


def build_program(bufs=3, psum_bufs=4):
    import concourse.mybir as mybir
    import concourse.tile as tile
    from concourse import bacc

    nc = bacc.Bacc()
    bf16 = mybir.dt.bfloat16
    f32 = mybir.dt.float32
    a_d = nc.dram_tensor("Ax", [KP, PPC, KC, 2, N], bf16, kind="ExternalInput")
    w_d = nc.dram_tensor("Wx", [KP, PPC, KC, 2, FOUT], bf16, kind="ExternalInput")
    b_d = nc.dram_tensor("bias", [FOUT], f32, kind="ExternalInput")
    z_d = nc.dram_tensor("Z", [FOUT, PPC, N], f32, kind="ExternalOutput")

    with tile.TileContext(nc) as tc, ExitStack() as ctx:
        wpool = ctx.enter_context(tc.tile_pool(name="w", bufs=bufs))
        apool = ctx.enter_context(tc.tile_pool(name="a", bufs=bufs))
        opool = ctx.enter_context(tc.tile_pool(name="o", bufs=bufs))
        pspool = ctx.enter_context(
            tc.tile_pool(name="ps", bufs=psum_bufs, space="PSUM")
        )
        singles = ctx.enter_context(tc.tile_pool(name="singles", bufs=1))

        bias_sb = singles.tile([FOUT, 1], f32)
        nc.sync.dma_start(out=bias_sb, in_=b_d[:, None])

        for g in range(NG):
            p0 = g * GP
            wx = wpool.tile([KP, GP, KC, 2, FOUT], bf16)
            nc.sync.dma_start(out=wx, in_=w_d[:, p0 : p0 + GP])
            ax = apool.tile([KP, GP, KC, 2, N], bf16)
            nc.sync.dma_start(out=ax, in_=a_d[:, p0 : p0 + GP])
            ot = opool.tile([FOUT, GP, N], f32)
            for j in range(GP):
                ps = pspool.tile([FOUT, N], f32)
                for kc in range(KC):
                    # psum[o, n] += Wh^T @ Ah + Wh^T @ Al + Wl^T @ Ah
                    nc.tensor.matmul(
                        ps,
                        wx[:, j, kc, 0, :],
                        ax[:, j, kc, 0, :],
                        start=(kc == 0),
                        stop=False,
                    )
                    nc.tensor.matmul(
                        ps,
                        wx[:, j, kc, 0, :],
                        ax[:, j, kc, 1, :],
                        start=False,
                        stop=False,
                    )
                    nc.tensor.matmul(
                        ps,
                        wx[:, j, kc, 1, :],
                        ax[:, j, kc, 0, :],
                        start=False,
                        stop=(kc == KC - 1),
                    )
                nc.scalar.activation(
                    ot[:, j, :],
                    ps,
                    mybir.ActivationFunctionType.Relu,
                    bias=bias_sb,
                )
            nc.sync.dma_start(out=z_d[:, p0 : p0 + GP, :], in_=ot)
    nc.finalize()
    return nc


def _split_bf16(x):
    """x (f32) -> (hi, lo) bf16 with hi + lo ~= x."""
    import ml_dtypes

    hi = x.astype(ml_dtypes.bfloat16)
    lo = (x - hi.astype(np.float32)).astype(ml_dtypes.bfloat16)
    return hi, lo


def shard_inputs(X, filters, bias):
    """Host-side reshape into per-core device layouts."""
    X = np.asarray(X, dtype=np.float32)
    filters = np.asarray(filters, dtype=np.float32)
    bias = np.ascontiguousarray(np.asarray(bias, dtype=np.float32))

    # X [n, h, w, c] -> patches [p, g, n] with g = (fh, fw, c) row-major
    xr = X.reshape(N, NR, FH, NCOL, FW, FIN)
    xp = xr.transpose(1, 3, 2, 4, 5, 0).reshape(P, K, N)
    # -> [core, k(128), p_local(32), kc(16), n]
    a_all = xp.reshape(N_CORES, PPC, KC, KP, N).transpose(0, 3, 1, 2, 4)
    ah, al = _split_bf16(a_all)
    ax = np.ascontiguousarray(np.stack([ah, al], axis=4))  # [c,128,32,16,2,64]

    # filters [p, fh, fw, c, o] -> [p, g, o] -> [core, k, p_local, kc, o]
    wp = filters.reshape(P, K, FOUT)
    w_all = wp.reshape(N_CORES, PPC, KC, KP, FOUT).transpose(0, 3, 1, 2, 4)
    wh, wl = _split_bf16(w_all)
    wx = np.ascontiguousarray(np.stack([wh, wl], axis=4))  # [c,128,32,16,2,128]

    return [{"Ax": ax[c], "Wx": wx[c], "bias": bias} for c in range(N_CORES)]


def gather_output(per_core_z):
    """per_core_z: list of 8 arrays [FOUT, PPC, N] -> [N, NR, NCOL, FOUT]."""
    z = np.stack(per_core_z, axis=0)  # [core, o, p, n]
    z = z.transpose(3, 0, 2, 1).reshape(N, P, FOUT)  # [n, p_global, o]
    return np.ascontiguousarray(z.reshape(N, NR, NCOL, FOUT))


def kernel(X, filters, bias):
    from concourse.bass_utils import run_bass_kernel_spmd

    if "nc" not in _PROGRAM_CACHE:
        _PROGRAM_CACHE["nc"] = build_program()
    nc = _PROGRAM_CACHE["nc"]

    in_maps = shard_inputs(X, filters, bias)
    res = run_bass_kernel_spmd(nc, in_maps, core_ids=list(range(N_CORES)))
    return gather_output([res.results[c]["Z"] for c in range(N_CORES)])


# revision 9
# speedup vs baseline: 1.9582x; 1.0317x over previous
"""Bass/Trainium2 kernel for nn_BioConvolution (per-patch unshared-filter conv).

Math: for each of P=256 non-overlapping 8x8 patches p,
    Z[n, p, :] = relu(A_p @ W_p + bias),  A_p = patches[n, p] in R^{64 x 2048},
    W_p in R^{2048 x 128}.

Sharding: patch dimension P across 8 cores (32 patches/core) -- every filter
and input pixel is read exactly once chip-wide (batch-parallel would
replicate the 256MB filter tensor 8x). The kernel is HBM-bound: ~50MB/core.

Precision: fp32 matmul on TRN2 lowers to 2 half-rate LOW_HIGH passes
(~434ns/chunk measured) which makes PE the bottleneck. Instead we use a
split-bf16 scheme: A = Ah + Al, W = Wh + Wl (bf16 each, exact fp32 sum of
products in PSUM): A*W ~= Ah*Wh + Al*Wh + Ah*Wl, dropping only Al*Wl
(~2^-18 relative). Measured rel err ~1e-5 vs fp32 reference. Same DMA bytes
as fp32, but full-rate bf16 matmuls.

Device layout (host pre-transposes; all DMAs are large and contiguous):
  Ax [128k, 32p, 16kc, 2, 64n]  bf16 -- patch pixels (hi/lo), k-major
  Wx [128k, 32p, 16kc, 2, 128o] bf16 -- filters (hi/lo), k-major
  Z  [128o, 32p, 64n] f32 -- output (host transposes back)
Per patch, one PSUM tile [128o, 2, 64n] accumulates 16 chunks x 2 matmuls:
  mmA: lhsT=Wh [128k,128o], rhs=[Ah|Al] [128k,128]  -> psum[:, :, :]
  mmB: lhsT=Wl [128k,128o], rhs=Ah [128k,64]        -> psum[:, 0, :]
Epilogue per patch: DVE add (hi+lo halves) then ScalarE Relu+bias.
"""

from contextlib import ExitStack

import numpy as np

N_CORES = 8
N, H, W_IMG, FIN = 64, 128, 128, 32
FH = FW = 8
FOUT = 128
NR, NCOL = H // FH, W_IMG // FW  # 16, 16
P = NR * NCOL  # 256
PPC = P // N_CORES  # 32 patches per core
K = FH * FW * FIN  # 2048
KP = 128  # contraction tile (partition dim)
KC = K // KP  # 16 chunks
GP = 2  # patches per DMA group
NG = PPC // GP  # 16 groups

_PROGRAM_CACHE = {}


def build_program(bufs=4, psum_bufs=4):
    import concourse.mybir as mybir
    import concourse.tile as tile
    from concourse import bacc

    nc = bacc.Bacc()
    bf16 = mybir.dt.bfloat16
    f32 = mybir.dt.float32
    a_d = nc.dram_tensor("Ax", [KP, PPC, KC, 2, N], bf16, kind="ExternalInput")
    w_d = nc.dram_tensor("Wx", [KP, PPC, KC, 2, FOUT], bf16, kind="ExternalInput")
    b_d = nc.dram_tensor("bias", [FOUT], f32, kind="ExternalInput")
    z_d = nc.dram_tensor("Z", [FOUT, PPC, N], f32, kind="ExternalOutput")

    with tile.TileContext(nc) as tc, ExitStack() as ctx:
        wpool = ctx.enter_context(tc.tile_pool(name="w", bufs=bufs))
        apool = ctx.enter_context(tc.tile_pool(name="a", bufs=bufs))
        opool = ctx.enter_context(tc.tile_pool(name="o", bufs=bufs))
        pspool = ctx.enter_context(
            tc.tile_pool(name="ps", bufs=psum_bufs, space="PSUM")
        )
        singles = ctx.enter_context(tc.tile_pool(name="singles", bufs=1))

        bias_sb = singles.tile([FOUT, 1], f32)
        nc.sync.dma_start(out=bias_sb, in_=b_d[:, None])

        for g in range(NG):
            p0 = g * GP
            wx = wpool.tile([KP, GP, KC, 2, FOUT], bf16)
            nc.sync.dma_start(out=wx, in_=w_d[:, p0 : p0 + GP])
            ax = apool.tile([KP, GP, KC, 2, N], bf16)
            nc.sync.dma_start(out=ax, in_=a_d[:, p0 : p0 + GP])
            ot = opool.tile([FOUT, GP, N], f32)
            for j in range(GP):
                ps = pspool.tile([FOUT, N], f32)
                for kc in range(KC):
                    # psum[o, n] += Wh^T @ Ah + Wh^T @ Al + Wl^T @ Ah
                    nc.tensor.matmul(
                        ps,
                        wx[:, j, kc, 0, :],
                        ax[:, j, kc, 0, :],
                        start=(kc == 0),
                        stop=False,
                    )
                    nc.tensor.matmul(
                        ps,
                        wx[:, j, kc, 0, :],
                        ax[:, j, kc, 1, :],
                        start=False,
                        stop=False,
                    )
                    nc.tensor.matmul(
                        ps,
                        wx[:, j, kc, 1, :],
                        ax[:, j, kc, 0, :],
                        start=False,
                        stop=(kc == KC - 1),
                    )
                nc.scalar.activation(
                    ot[:, j, :],
                    ps,
                    mybir.ActivationFunctionType.Relu,
                    bias=bias_sb,
                )
            # ACT HWDGE ring: keeps the store off the SP input-DMA FIFO
            # (head-of-line blocking of the next group's loads).
            nc.scalar.dma_start(out=z_d[:, p0 : p0 + GP, :], in_=ot)
    nc.finalize()
    return nc


def _split_bf16(x):
    """x (f32) -> (hi, lo) bf16 with hi + lo ~= x."""
    import ml_dtypes

    hi = x.astype(ml_dtypes.bfloat16)
    lo = (x - hi.astype(np.float32)).astype(ml_dtypes.bfloat16)
    return hi, lo


def shard_inputs(X, filters, bias):
    """Host-side reshape into per-core device layouts."""
    X = np.asarray(X, dtype=np.float32)
    filters = np.asarray(filters, dtype=np.float32)
    bias = np.ascontiguousarray(np.asarray(bias, dtype=np.float32))

    # X [n, h, w, c] -> patches [p, g, n] with g = (fh, fw, c) row-major
    xr = X.reshape(N, NR, FH, NCOL, FW, FIN)
    xp = xr.transpose(1, 3, 2, 4, 5, 0).reshape(P, K, N)
    # -> [core, k(128), p_local(32), kc(16), n]
    a_all = xp.reshape(N_CORES, PPC, KC, KP, N).transpose(0, 3, 1, 2, 4)
    ah, al = _split_bf16(a_all)
    ax = np.ascontiguousarray(np.stack([ah, al], axis=4))  # [c,128,32,16,2,64]

    # filters [p, fh, fw, c, o] -> [p, g, o] -> [core, k, p_local, kc, o]
    wp = filters.reshape(P, K, FOUT)
    w_all = wp.reshape(N_CORES, PPC, KC, KP, FOUT).transpose(0, 3, 1, 2, 4)
    wh, wl = _split_bf16(w_all)
    wx = np.ascontiguousarray(np.stack([wh, wl], axis=4))  # [c,128,32,16,2,128]

    return [{"Ax": ax[c], "Wx": wx[c], "bias": bias} for c in range(N_CORES)]


def gather_output(per_core_z):
    """per_core_z: list of 8 arrays [FOUT, PPC, N] -> [N, NR, NCOL, FOUT]."""
    z = np.stack(per_core_z, axis=0)  # [core, o, p, n]
    z = z.transpose(3, 0, 2, 1).reshape(N, P, FOUT)  # [n, p_global, o]
    return np.ascontiguousarray(z.reshape(N, NR, NCOL, FOUT))


def kernel(X, filters, bias):
    from concourse.bass_utils import run_bass_kernel_spmd

    if "nc" not in _PROGRAM_CACHE:
        _PROGRAM_CACHE["nc"] = build_program()
    nc = _PROGRAM_CACHE["nc"]

    in_maps = shard_inputs(X, filters, bias)
    res = run_bass_kernel_spmd(nc, in_maps, core_ids=list(range(N_CORES)))
    return gather_output([res.results[c]["Z"] for c in range(N_CORES)])


# revision 10
# speedup vs baseline: 2.0183x; 1.0307x over previous
"""Bass/Trainium2 kernel for nn_BioConvolution (per-patch unshared-filter conv).

Math: for each of P=256 non-overlapping 8x8 patches p,
    Z[n, p, :] = relu(A_p @ W_p + bias),  A_p = patches[n, p] in R^{64 x 2048},
    W_p in R^{2048 x 128}.

Sharding: patch dimension P across 8 cores (32 patches/core) -- every filter
and input pixel is read exactly once chip-wide (batch-parallel would
replicate the 256MB filter tensor 8x). The kernel is HBM-bound: ~50MB/core.

Precision: fp32 matmul on TRN2 lowers to 2 half-rate LOW_HIGH passes
(~434ns/chunk measured) which makes PE the bottleneck. Instead we use a
split-bf16 scheme: A = Ah + Al, W = Wh + Wl (bf16 each, exact fp32 sum of
products in PSUM): A*W ~= Ah*Wh + Al*Wh + Ah*Wl, dropping only Al*Wl
(~2^-18 relative). Measured rel err ~1e-5 vs fp32 reference. Same DMA bytes
as fp32, but full-rate bf16 matmuls.

Device layout (host pre-transposes; all DMAs are large and contiguous):
  Ax [128k, 32p, 16kc, 2, 64n]  bf16 -- patch pixels (hi/lo), k-major
  Wx [128k, 32p, 16kc, 2, 128o] bf16 -- filters (hi/lo), k-major
  Z  [128o, 32p, 64n] f32 -- output (host transposes back)
Per patch, one PSUM tile [128o, 2, 64n] accumulates 16 chunks x 2 matmuls:
  mmA: lhsT=Wh [128k,128o], rhs=[Ah|Al] [128k,128]  -> psum[:, :, :]
  mmB: lhsT=Wl [128k,128o], rhs=Ah [128k,64]        -> psum[:, 0, :]
Epilogue per patch: DVE add (hi+lo halves) then ScalarE Relu+bias.
"""

from contextlib import ExitStack

import numpy as np

N_CORES = 8
N, H, W_IMG, FIN = 64, 128, 128, 32
FH = FW = 8
FOUT = 128
NR, NCOL = H // FH, W_IMG // FW  # 16, 16
P = NR * NCOL  # 256
PPC = P // N_CORES  # 32 patches per core
K = FH * FW * FIN  # 2048
KP = 128  # contraction tile (partition dim)
KC = K // KP  # 16 chunks
GP = 4  # patches per DMA group
NG = PPC // GP  # 8 groups

_PROGRAM_CACHE = {}


def build_program(bufs=3, psum_bufs=4):
    import concourse.mybir as mybir
    import concourse.tile as tile
    from concourse import bacc

    nc = bacc.Bacc()
    bf16 = mybir.dt.bfloat16
    f32 = mybir.dt.float32
    a_d = nc.dram_tensor("Ax", [KP, PPC, KC, 2, N], bf16, kind="ExternalInput")
    w_d = nc.dram_tensor("Wx", [KP, PPC, KC, 2, FOUT], bf16, kind="ExternalInput")
    b_d = nc.dram_tensor("bias", [FOUT], f32, kind="ExternalInput")
    z_d = nc.dram_tensor("Z", [FOUT, PPC, N], f32, kind="ExternalOutput")

    with tile.TileContext(nc) as tc, ExitStack() as ctx:
        wpool = ctx.enter_context(tc.tile_pool(name="w", bufs=bufs))
        apool = ctx.enter_context(tc.tile_pool(name="a", bufs=bufs))
        opool = ctx.enter_context(tc.tile_pool(name="o", bufs=bufs))
        pspool = ctx.enter_context(
            tc.tile_pool(name="ps", bufs=psum_bufs, space="PSUM")
        )
        singles = ctx.enter_context(tc.tile_pool(name="singles", bufs=1))

        bias_sb = singles.tile([FOUT, 1], f32)
        nc.sync.dma_start(out=bias_sb, in_=b_d[:, None])

        for g in range(NG):
            p0 = g * GP
            wx = wpool.tile([KP, GP, KC, 2, FOUT], bf16)
            nc.sync.dma_start(out=wx, in_=w_d[:, p0 : p0 + GP])
            ax = apool.tile([KP, GP, KC, 2, N], bf16)
            nc.sync.dma_start(out=ax, in_=a_d[:, p0 : p0 + GP])
            ot = opool.tile([FOUT, GP, N], f32)
            for j in range(GP):
                ps = pspool.tile([FOUT, N], f32)
                for kc in range(KC):
                    # psum[o, n] += Wh^T @ Ah + Wh^T @ Al + Wl^T @ Ah
                    nc.tensor.matmul(
                        ps,
                        wx[:, j, kc, 0, :],
                        ax[:, j, kc, 0, :],
                        start=(kc == 0),
                        stop=False,
                    )
                    nc.tensor.matmul(
                        ps,
                        wx[:, j, kc, 0, :],
                        ax[:, j, kc, 1, :],
                        start=False,
                        stop=False,
                    )
                    nc.tensor.matmul(
                        ps,
                        wx[:, j, kc, 1, :],
                        ax[:, j, kc, 0, :],
                        start=False,
                        stop=(kc == KC - 1),
                    )
                nc.scalar.activation(
                    ot[:, j, :],
                    ps,
                    mybir.ActivationFunctionType.Relu,
                    bias=bias_sb,
                )
            # ACT HWDGE ring: keeps the store off the SP input-DMA FIFO
            # (head-of-line blocking of the next group's loads).
            nc.scalar.dma_start(out=z_d[:, p0 : p0 + GP, :], in_=ot)
    nc.finalize()
    return nc


def _split_bf16(x):
    """x (f32) -> (hi, lo) bf16 with hi + lo ~= x."""
    import ml_dtypes

    hi = x.astype(ml_dtypes.bfloat16)
    lo = (x - hi.astype(np.float32)).astype(ml_dtypes.bfloat16)
    return hi, lo


def shard_inputs(X, filters, bias):
    """Host-side reshape into per-core device layouts."""
    X = np.asarray(X, dtype=np.float32)
    filters = np.asarray(filters, dtype=np.float32)
    bias = np.ascontiguousarray(np.asarray(bias, dtype=np.float32))

    # X [n, h, w, c] -> patches [p, g, n] with g = (fh, fw, c) row-major
    xr = X.reshape(N, NR, FH, NCOL, FW, FIN)
    xp = xr.transpose(1, 3, 2, 4, 5, 0).reshape(P, K, N)
    # -> [core, k(128), p_local(32), kc(16), n]
    a_all = xp.reshape(N_CORES, PPC, KC, KP, N).transpose(0, 3, 1, 2, 4)
    ah, al = _split_bf16(a_all)
    ax = np.ascontiguousarray(np.stack([ah, al], axis=4))  # [c,128,32,16,2,64]

    # filters [p, fh, fw, c, o] -> [p, g, o] -> [core, k, p_local, kc, o]
    wp = filters.reshape(P, K, FOUT)
    w_all = wp.reshape(N_CORES, PPC, KC, KP, FOUT).transpose(0, 3, 1, 2, 4)
    wh, wl = _split_bf16(w_all)
    wx = np.ascontiguousarray(np.stack([wh, wl], axis=4))  # [c,128,32,16,2,128]

    return [{"Ax": ax[c], "Wx": wx[c], "bias": bias} for c in range(N_CORES)]


def gather_output(per_core_z):
    """per_core_z: list of 8 arrays [FOUT, PPC, N] -> [N, NR, NCOL, FOUT]."""
    z = np.stack(per_core_z, axis=0)  # [core, o, p, n]
    z = z.transpose(3, 0, 2, 1).reshape(N, P, FOUT)  # [n, p_global, o]
    return np.ascontiguousarray(z.reshape(N, NR, NCOL, FOUT))


def kernel(X, filters, bias):
    from concourse.bass_utils import run_bass_kernel_spmd

    if "nc" not in _PROGRAM_CACHE:
        _PROGRAM_CACHE["nc"] = build_program()
    nc = _PROGRAM_CACHE["nc"]

    in_maps = shard_inputs(X, filters, bias)
    res = run_bass_kernel_spmd(nc, in_maps, core_ids=list(range(N_CORES)))
    return gather_output([res.results[c]["Z"] for c in range(N_CORES)])


# revision 12
# speedup vs baseline: 2.4345x; 1.2062x over previous
"""Bass/Trainium2 kernel for nn_BioConvolution (per-patch unshared-filter conv).

Math: for each of P=256 non-overlapping 8x8 patches p,
    Z[n, p, :] = relu(A_p @ W_p + bias),  A_p = patches[n, p] in R^{64 x 2048},
    W_p in R^{2048 x 128}.

Sharding: patch dimension P across 8 cores (32 patches/core) -- every filter
and input pixel is read exactly once chip-wide (batch-parallel would
replicate the 256MB filter tensor 8x). The kernel is HBM-bound: ~50MB/core.

Precision: fp32 matmul on TRN2 lowers to 2 half-rate LOW_HIGH passes
(~434ns/chunk measured) which makes PE the bottleneck. Instead we use a
split-bf16 scheme: A = Ah + Al, W = Wh + Wl (bf16 each, exact fp32 sum of
products in PSUM): A*W ~= Ah*Wh + Al*Wh + Ah*Wl, dropping only Al*Wl
(~2^-18 relative). Measured rel err ~1e-5 vs fp32 reference. Same DMA bytes
as fp32, but full-rate bf16 matmuls.

Device layout (host pre-transposes; all DMAs are large and contiguous):
  Ax [128k, 32p, 16kc, 2, 64n]  bf16 -- patch pixels (hi/lo), k-major
  Wx [128k, 32p, 16kc, 2, 128o] bf16 -- filters (hi/lo), k-major
  Z  [128o, 32p, 64n] f32 -- output (host transposes back)
Per patch, one PSUM tile [128o, 2, 64n] accumulates 16 chunks x 2 matmuls:
  mmA: lhsT=Wh [128k,128o], rhs=[Ah|Al] [128k,128]  -> psum[:, :, :]
  mmB: lhsT=Wl [128k,128o], rhs=Ah [128k,64]        -> psum[:, 0, :]
Epilogue per patch: DVE add (hi+lo halves) then ScalarE Relu+bias.
"""

from contextlib import ExitStack

import numpy as np

N_CORES = 8
N, H, W_IMG, FIN = 64, 128, 128, 32
FH = FW = 8
FOUT = 128
NR, NCOL = H // FH, W_IMG // FW  # 16, 16
P = NR * NCOL  # 256
PPC = P // N_CORES  # 32 patches per core
K = FH * FW * FIN  # 2048
KP = 128  # contraction tile (partition dim)
KC = K // KP  # 16 chunks
GP = 4  # patches per DMA group
NG = PPC // GP  # 8 groups

_PROGRAM_CACHE = {}


def build_program(bufs=3, psum_bufs=4):
    import concourse.mybir as mybir
    import concourse.tile as tile
    from concourse import bacc

    nc = bacc.Bacc()
    bf16 = mybir.dt.bfloat16
    f32 = mybir.dt.float32
    a_d = nc.dram_tensor("Ax", [KP, PPC, KC, 2, N], bf16, kind="ExternalInput")
    w_d = nc.dram_tensor("Wx", [KP, PPC, KC, 2, FOUT], bf16, kind="ExternalInput")
    b_d = nc.dram_tensor("bias", [FOUT], f32, kind="ExternalInput")
    z_d = nc.dram_tensor("Z", [FOUT, PPC, N], f32, kind="ExternalOutput")

    with tile.TileContext(nc) as tc, ExitStack() as ctx:
        wpool = ctx.enter_context(tc.tile_pool(name="w", bufs=bufs))
        apool = ctx.enter_context(tc.tile_pool(name="a", bufs=bufs))
        opool = ctx.enter_context(tc.tile_pool(name="o", bufs=bufs))
        pspool = ctx.enter_context(
            tc.tile_pool(name="ps", bufs=psum_bufs, space="PSUM")
        )
        singles = ctx.enter_context(tc.tile_pool(name="singles", bufs=1))

        bias_sb = singles.tile([FOUT, 1], f32)
        # SWDGE: keep this tiny load off the SP HWDGE FIFO head, where its
        # 128 4-byte descriptors would delay the first weight load.
        nc.gpsimd.dma_start(out=bias_sb, in_=b_d[:, None])

        # Last group split small so the final compute+store drain is short.
        group_sizes = [GP] * (NG - 1) + [GP // 2, GP // 2]
        p0 = 0
        for gp in group_sizes:
            wx = wpool.tile([KP, gp, KC, 2, FOUT], bf16, tag="wx")
            nc.sync.dma_start(out=wx, in_=w_d[:, p0 : p0 + gp])
            ax = apool.tile([KP, gp, KC, 2, N], bf16, tag="ax")
            nc.sync.dma_start(out=ax, in_=a_d[:, p0 : p0 + gp])
            ot = opool.tile([FOUT, gp, N], f32, tag="ot")
            for j in range(gp):
                ps = pspool.tile([FOUT, N], f32)
                for kc in range(KC):
                    # psum[o, n] += Wh^T @ Ah + Wh^T @ Al + Wl^T @ Ah
                    nc.tensor.matmul(
                        ps,
                        wx[:, j, kc, 0, :],
                        ax[:, j, kc, 0, :],
                        start=(kc == 0),
                        stop=False,
                    )
                    nc.tensor.matmul(
                        ps,
                        wx[:, j, kc, 0, :],
                        ax[:, j, kc, 1, :],
                        start=False,
                        stop=False,
                    )
                    nc.tensor.matmul(
                        ps,
                        wx[:, j, kc, 1, :],
                        ax[:, j, kc, 0, :],
                        start=False,
                        stop=(kc == KC - 1),
                    )
                nc.scalar.activation(
                    ot[:, j, :],
                    ps,
                    mybir.ActivationFunctionType.Relu,
                    bias=bias_sb,
                )
            # ACT HWDGE ring: keeps the store off the SP input-DMA FIFO
            # (head-of-line blocking of the next group's loads).
            nc.scalar.dma_start(out=z_d[:, p0 : p0 + gp, :], in_=ot)
            p0 += gp
    nc.finalize()
    return nc


def _split_bf16(x):
    """x (f32) -> (hi, lo) bf16 with hi + lo ~= x."""
    import ml_dtypes

    hi = x.astype(ml_dtypes.bfloat16)
    lo = (x - hi.astype(np.float32)).astype(ml_dtypes.bfloat16)
    return hi, lo


def shard_inputs(X, filters, bias):
    """Host-side reshape into per-core device layouts."""
    X = np.asarray(X, dtype=np.float32)
    filters = np.asarray(filters, dtype=np.float32)
    bias = np.ascontiguousarray(np.asarray(bias, dtype=np.float32))

    # X [n, h, w, c] -> patches [p, g, n] with g = (fh, fw, c) row-major
    xr = X.reshape(N, NR, FH, NCOL, FW, FIN)
    xp = xr.transpose(1, 3, 2, 4, 5, 0).reshape(P, K, N)
    # -> [core, k(128), p_local(32), kc(16), n]
    a_all = xp.reshape(N_CORES, PPC, KC, KP, N).transpose(0, 3, 1, 2, 4)
    ah, al = _split_bf16(a_all)
    ax = np.ascontiguousarray(np.stack([ah, al], axis=4))  # [c,128,32,16,2,64]

    # filters [p, fh, fw, c, o] -> [p, g, o] -> [core, k, p_local, kc, o]
    wp = filters.reshape(P, K, FOUT)
    w_all = wp.reshape(N_CORES, PPC, KC, KP, FOUT).transpose(0, 3, 1, 2, 4)
    wh, wl = _split_bf16(w_all)
    wx = np.ascontiguousarray(np.stack([wh, wl], axis=4))  # [c,128,32,16,2,128]

    return [{"Ax": ax[c], "Wx": wx[c], "bias": bias} for c in range(N_CORES)]


def gather_output(per_core_z):
    """per_core_z: list of 8 arrays [FOUT, PPC, N] -> [N, NR, NCOL, FOUT]."""
    z = np.stack(per_core_z, axis=0)  # [core, o, p, n]
    z = z.transpose(3, 0, 2, 1).reshape(N, P, FOUT)  # [n, p_global, o]
    return np.ascontiguousarray(z.reshape(N, NR, NCOL, FOUT))


def kernel(X, filters, bias):
    from concourse.bass_utils import run_bass_kernel_spmd

    if "nc" not in _PROGRAM_CACHE:
        _PROGRAM_CACHE["nc"] = build_program()
    nc = _PROGRAM_CACHE["nc"]

    in_maps = shard_inputs(X, filters, bias)
    res = run_bass_kernel_spmd(nc, in_maps, core_ids=list(range(N_CORES)))
    return gather_output([res.results[c]["Z"] for c in range(N_CORES)])


# revision 14
# speedup vs baseline: 2.4689x; 1.0141x over previous
"""fp16+fp8 variant: 3 bytes/element instead of 4 -> ~25% less HBM traffic.

A = Ah(fp16) + Al/S (fp8e4m3 scaled by S=2^15), same for W. Then
  A*W ~= Ah*Wh + (Al'*Wh + Ah*Wl')/S    (drop Al*Wl ~ 2^-24)
fp16 products are exact in fp32 PSUM; fp8 quantization of the residual gives
~1.2e-5 max rel err (validated in numpy). fp16 subnormals are flushed on the
host so device FTZ cannot diverge.

Two PSUM groups per patch: psM (16 MMs) and psR (32 MMs, shared scale S).
Epilogue: t = psR * (1/S) on ACT, u = psM + t on DVE, relu(u + bias) on ACT.
Residual fp8 tensors are loaded raw over HWDGE and upconverted fp8->fp16 by
DVE tensor_copy (SWDGE cast-during-DMA measured 2x slower).
Measured: 130.8us HW exec, rel err 1.13e-5 (vs 157us at 4.9e-6 for the
bf16 hi/lo 4-byte variant, kept in kernel_splitbf16_157us.py.bak).
"""

from contextlib import ExitStack

import numpy as np

N_CORES = 8
N, H, W_IMG, FIN = 64, 128, 128, 32
FH = FW = 8
FOUT = 128
NR, NCOL = H // FH, W_IMG // FW
P = NR * NCOL
PPC = P // N_CORES
K = FH * FW * FIN
KP = 128
KC = K // KP
GP = 4
NG = PPC // GP
RSCALE = 2.0**15

_PROGRAM_CACHE = {}


def build_program(bufs=3):
    import concourse.mybir as mybir
    import concourse.tile as tile
    from concourse import bacc

    nc = bacc.Bacc()
    f16 = mybir.dt.float16
    f8 = mybir.dt.float8e4
    f32 = mybir.dt.float32
    ah_d = nc.dram_tensor("Ah", [KP, PPC, KC, N], f16, kind="ExternalInput")
    wh_d = nc.dram_tensor("Wh", [KP, PPC, KC, FOUT], f16, kind="ExternalInput")
    ar_d = nc.dram_tensor("Ar", [KP, PPC, KC, N], f8, kind="ExternalInput")
    wr_d = nc.dram_tensor("Wr", [KP, PPC, KC, FOUT], f8, kind="ExternalInput")
    b_d = nc.dram_tensor("bias", [FOUT], f32, kind="ExternalInput")
    z_d = nc.dram_tensor("Z", [FOUT, PPC, N], f32, kind="ExternalOutput")

    with tile.TileContext(nc) as tc, ExitStack() as ctx:
        whp = ctx.enter_context(tc.tile_pool(name="wh", bufs=bufs))
        ahp = ctx.enter_context(tc.tile_pool(name="ah", bufs=bufs))
        wrp = ctx.enter_context(tc.tile_pool(name="wr", bufs=bufs))
        arp = ctx.enter_context(tc.tile_pool(name="ar", bufs=bufs))
        wr8p = ctx.enter_context(tc.tile_pool(name="wr8", bufs=bufs))
        ar8p = ctx.enter_context(tc.tile_pool(name="ar8", bufs=bufs))
        opool = ctx.enter_context(tc.tile_pool(name="o", bufs=bufs))
        tpool = ctx.enter_context(tc.tile_pool(name="t", bufs=4))
        psm = ctx.enter_context(tc.tile_pool(name="psm", bufs=3, space="PSUM"))
        psr = ctx.enter_context(tc.tile_pool(name="psr", bufs=3, space="PSUM"))
        singles = ctx.enter_context(tc.tile_pool(name="singles", bufs=1))

        bias_sb = singles.tile([FOUT, 1], f32)
        nc.gpsimd.dma_start(out=bias_sb, in_=b_d[:, None])

        group_sizes = [GP] * (NG - 1) + [GP // 2, GP // 2]
        p0 = 0
        for gp in group_sizes:
            wh = whp.tile([KP, gp, KC, FOUT], f16, tag="wh")
            nc.sync.dma_start(out=wh, in_=wh_d[:, p0 : p0 + gp])
            ah = ahp.tile([KP, gp, KC, N], f16, tag="ah")
            nc.sync.dma_start(out=ah, in_=ah_d[:, p0 : p0 + gp])
            # residuals: fp8 in HBM via HWDGE, upconvert fp8->fp16 on DVE
            # (SWDGE cast-DMA measured 2x slower: Q7 descriptor gen + engine
            # contention; DVE is otherwise idle here)
            wr8 = wr8p.tile([KP, gp, KC, FOUT], f8, tag="wr8")
            nc.sync.dma_start(out=wr8, in_=wr_d[:, p0 : p0 + gp])
            ar8 = ar8p.tile([KP, gp, KC, N], f8, tag="ar8")
            nc.sync.dma_start(out=ar8, in_=ar_d[:, p0 : p0 + gp])
            wr = wrp.tile([KP, gp, KC, FOUT], f16, tag="wr")
            nc.vector.tensor_copy(wr, wr8)
            ar = arp.tile([KP, gp, KC, N], f16, tag="ar")
            nc.vector.tensor_copy(ar, ar8)

            ot = opool.tile([FOUT, gp, N], f32, tag="ot")
            for j in range(gp):
                psum_m = psm.tile([FOUT, N], f32, tag="psm")
                psum_r = psr.tile([FOUT, N], f32, tag="psr")
                for kc in range(KC):
                    nc.tensor.matmul(
                        psum_m,
                        wh[:, j, kc, :],
                        ah[:, j, kc, :],
                        start=(kc == 0),
                        stop=(kc == KC - 1),
                    )
                    nc.tensor.matmul(
                        psum_r,
                        wh[:, j, kc, :],
                        ar[:, j, kc, :],
                        start=(kc == 0),
                        stop=False,
                    )
                    nc.tensor.matmul(
                        psum_r,
                        wr[:, j, kc, :],
                        ah[:, j, kc, :],
                        start=False,
                        stop=(kc == KC - 1),
                    )
                tsum = tpool.tile([FOUT, N], f32, tag="tsum")
                nc.scalar.activation(
                    tsum,
                    psum_r,
                    mybir.ActivationFunctionType.Copy,
                    scale=float(1.0 / RSCALE),
                )
                usum = tpool.tile([FOUT, N], f32, tag="usum")
                nc.vector.tensor_add(usum, psum_m, tsum)
                nc.scalar.activation(
                    ot[:, j, :],
                    usum,
                    mybir.ActivationFunctionType.Relu,
                    bias=bias_sb,
                )
            nc.scalar.dma_start(out=z_d[:, p0 : p0 + gp, :], in_=ot)
            p0 += gp
    nc.finalize()
    return nc


def _split_fp16_fp8(x):
    import ml_dtypes

    hi = x.astype(np.float16)
    # flush fp16 subnormals so device FTZ matches the host residual
    hi = np.where(np.abs(hi.astype(np.float32)) < 6.104e-5, np.float16(0), hi)
    lo = ((x - hi.astype(np.float32)) * np.float32(RSCALE)).astype(
        ml_dtypes.float8_e4m3
    )
    return hi, lo


def shard_inputs(X, filters, bias):
    X = np.asarray(X, dtype=np.float32)
    filters = np.asarray(filters, dtype=np.float32)
    bias = np.ascontiguousarray(np.asarray(bias, dtype=np.float32))

    xr = X.reshape(N, NR, FH, NCOL, FW, FIN)
    xp = xr.transpose(1, 3, 2, 4, 5, 0).reshape(P, K, N)
    a_all = np.ascontiguousarray(
        xp.reshape(N_CORES, PPC, KC, KP, N).transpose(0, 3, 1, 2, 4)
    )
    ah, ar = _split_fp16_fp8(a_all)

    wp = filters.reshape(P, K, FOUT)
    w_all = np.ascontiguousarray(
        wp.reshape(N_CORES, PPC, KC, KP, FOUT).transpose(0, 3, 1, 2, 4)
    )
    wh, wr = _split_fp16_fp8(w_all)

    return [
        {"Ah": ah[c], "Wh": wh[c], "Ar": ar[c], "Wr": wr[c], "bias": bias}
        for c in range(N_CORES)
    ]


def gather_output(per_core_z):
    z = np.stack(per_core_z, axis=0)
    z = z.transpose(3, 0, 2, 1).reshape(N, P, FOUT)
    return np.ascontiguousarray(z.reshape(N, NR, NCOL, FOUT))


def kernel(X, filters, bias):
    from concourse.bass_utils import run_bass_kernel_spmd

    if "nc" not in _PROGRAM_CACHE:
        _PROGRAM_CACHE["nc"] = build_program()
    nc = _PROGRAM_CACHE["nc"]

    in_maps = shard_inputs(X, filters, bias)
    res = run_bass_kernel_spmd(nc, in_maps, core_ids=list(range(N_CORES)))
    return gather_output([res.results[c]["Z"] for c in range(N_CORES)])
